# revision 1
# baseline (speedup 1.0000x reference)
"""Patch TileContext._drain_and_barrier: this container's walrus codegen
rejects >2 sem waits on one CTRL (Drain) instruction. Split the kernel-tail
drain's waits across separate nop instructions (1 wait each)."""
import concourse.tile as tile  # noqa
import concourse.mybir as mybir
from concourse.vector_clock import ScopedClock
from concourse._compat import not_none as nn


def _drain_and_barrier_split(self, tick_clock, wait_clock):
    nc = self.nc
    carrier = nc.sync.nop()
    wait_clock.add_sem_waits(carrier.ins, ScopedClock({None: tick_clock.global_clock}))
    si = carrier.ins.sync_info
    waits = list(si.on_wait) if si and si.on_wait else []
    if len(waits) > 1:
        si.on_wait.clear()
        si.on_wait.append(waits[0])
        for w in waits[1:]:
            n2 = nc.sync.nop()
            n2.ins.sync_info = mybir.SyncInfo(on_wait=[w], on_update=[])
    nc.sync.drain()

    nc.all_engine_barrier()
    assert self.sems is not None
    popped = nc._tile_sem_poison_stack.pop()
    assert popped is self._sem_poison
    nc.clear_and_free_semaphores(list(self.sems.allocated().values()))
    nc.all_engine_barrier()


tile.TileContext._drain_and_barrier = _drain_and_barrier_split


# ---- global wait-splitting pass ----
# This walrus build packs at most MAX_WAITS sem-waits per instruction
# (ISA EVENTS struct holds one; codegen can prepend a limited number of
# sync-wait commands). Move excess waits onto InstNoOp carriers.
MAX_WAITS = 2

def fix_waits(nc, max_waits=MAX_WAITS):
    import concourse.mybir as mybir
    dma2 = getattr(nc, "_fix_dma_waits2", False)
    n_fixed = 0
    for fn in nc.m.functions:
        for blk in fn.blocks:
            insts = blk.instructions
            out = []
            for inst in insts:
                lim = max_waits
                if dma2 and isinstance(inst, mybir.InstDMACopy):
                    lim = 2
                si = getattr(inst, "sync_info", None)
                if si is not None and si.on_wait and len(si.on_wait) > lim:
                    waits = list(si.on_wait)
                    si.on_wait.clear()
                    for w in waits[:-lim] if lim else waits:
                        n_fixed += 1
                        nop = mybir.InstNoOp(
                            name=f"{inst.name}.wsplit{n_fixed}",
                            sync_info=mybir.SyncInfo(on_wait=[w], on_update=[]),
                            bass_nofuse=True,
                            engine=inst.engine,
                        )
                        out.append(nop)
                    for w in waits[-lim:] if lim else []:
                        si.on_wait.append(w)
                elif si is not None and si.on_wait and len(si.on_wait) > 1 and getattr(inst, "opcode", None) is None:
                    pass
                out.append(inst)
            blk.instructions = out
    return n_fixed


# auto-apply fix_waits on serialization
import concourse.bass as _bass
_orig_to_json_bytes = _bass.Bass.to_json_bytes

def _to_json_bytes_fixed(self, *a, **kw):
    try:
        fix_waits(self, max_waits=getattr(self, "_fix_max_waits", 1))
    except Exception as e:
        import traceback; traceback.print_exc()
    return _orig_to_json_bytes(self, *a, **kw)

_bass.Bass.to_json_bytes = _to_json_bytes_fixed


"""NodeModel GNN kernel for Trainium2 (Bass/Tile), 8-core SPMD.

Strategy:
- Shard destination NODES into 8 contiguous ranges of 6250; each core handles
  exactly the edges targeting its nodes (no collectives needed).
- Per core, group destination nodes by in-degree (descending). A shared
  schedule (degree-histogram max across cores) makes the program identical on
  all cores (SPMD); cores with fewer nodes of a given degree use virtual
  columns whose results scatter to a dump row.
- Columns = destination nodes; round r processes the r-th edge of each still-
  active column (active widths shrink as degree decreases -> zero padding).
- Segment SUM falls out of PSUM matmul accumulation across rounds; MAX/MIN are
  running DVE tensor-tensor ops on the per-round h3; COUNT is the degree
  (host-known, shipped as reciprocal for the mean).
- Gathers/scatters via gpsimd indirect DMA (128 rows/instruction).
"""

import numpy as np

import concourse.bass as bass
import concourse.mybir as mybir
import concourse.tile as tile
from concourse.bass import IndirectOffsetOnAxis

F32 = mybir.dt.float32
I32 = mybir.dt.int32
AF = mybir.ActivationFunctionType
ALU = mybir.AluOpType

P = 128
W = 512  # tile width (columns = destination nodes)


def build_schedule(col, n_nodes, n_cores, in_ch, lat_ch):
    """Host-side index preprocessing. Returns shared schedule + per-core arrays."""
    ncore_nodes = n_nodes // n_cores
    deg_all = np.bincount(col, minlength=n_nodes)
    dmax = int(deg_all.max())

    # per-core degree histograms of own nodes
    hist = np.zeros((n_cores, dmax + 1), np.int64)
    for c in range(n_cores):
        d = deg_all[c * ncore_nodes : (c + 1) * ncore_nodes]
        hist[c] = np.bincount(d, minlength=dmax + 1)
    H = hist.max(axis=0)  # shared histogram (per exact degree), index 0 unused

    # shared column degree sequence, descending
    col_degs = np.repeat(np.arange(dmax, 0, -1), H[dmax:0:-1])
    n_cols = len(col_degs)
    n_tiles = (n_cols + W - 1) // W

    # CSR of edges by destination (stable order)
    order = np.argsort(col, kind="stable")
    starts = np.zeros(n_nodes + 1, np.int64)
    np.cumsum(deg_all, out=starts[1:])

    # per-core: map shared columns -> node ids (real) or -1 (virtual)
    col_node = np.full((n_cores, n_cols), -1, np.int64)
    for c in range(n_cores):
        d_own = deg_all[c * ncore_nodes : (c + 1) * ncore_nodes]
        nodes_by_deg = {}
        for ln in np.argsort(-d_own, kind="stable"):
            if d_own[ln] == 0:
                break
            nodes_by_deg.setdefault(int(d_own[ln]), []).append(ln)
        used = {d: 0 for d in range(1, dmax + 1)}
        for j in range(n_cols):
            d = int(col_degs[j])
            lst = nodes_by_deg.get(d, [])
            k = used[d]
            if k < len(lst):
                col_node[c, j] = c * ncore_nodes + lst[k]
                used[d] = k + 1

    # schedule: per tile, list of round widths
    tiles = []
    for t in range(n_tiles):
        j0, j1 = t * W, min((t + 1) * W, n_cols)
        degs = col_degs[j0:j1]
        d_t = int(degs[0])
        widths = [int(np.searchsorted(-degs, -(r + 1), side="right")) for r in range(d_t)]
        tiles.append((j0, j1, widths))

    # per-core slot gather indices (row = source of r-th edge of column node)
    n_chunk_slots = sum(sum((w + P - 1) // P for w in widths) for _, _, widths in tiles)
    n_col_chunks = sum((j1 - j0 + P - 1) // P for j0, j1, _ in tiles)

    idx_row = np.zeros((n_cores, P, n_chunk_slots), np.int32)
    idx_col = np.zeros((n_cores, P, n_col_chunks), np.int32)
    idx_scat = np.full((n_cores, P, n_col_chunks), ncore_nodes, np.int32)  # dump row
    rdeg = np.zeros((n_cores, 64, n_col_chunks * P), np.float32)

    row_sorted = None  # filled by caller loop below

    return dict(
        ncore_nodes=ncore_nodes, deg_all=deg_all, H=H, col_degs=col_degs,
        n_cols=n_cols, n_tiles=n_tiles, tiles=tiles, order=order, starts=starts,
        col_node=col_node, idx_row=idx_row, idx_col=idx_col, idx_scat=idx_scat,
        rdeg=rdeg, n_chunk_slots=n_chunk_slots, n_col_chunks=n_col_chunks,
        dmax=dmax,
    )


def fill_indices(sched, row, col, n_cores):
    """Fill per-core gather/scatter index arrays following the tile schedule."""
    order, starts = sched["order"], sched["starts"]
    col_node = sched["col_node"]
    tiles = sched["tiles"]
    ncore_nodes = sched["ncore_nodes"]
    deg_all = sched["deg_all"]
    idx_row, idx_col, idx_scat, rdeg = (
        sched["idx_row"], sched["idx_col"], sched["idx_scat"], sched["rdeg"])

    for c in range(n_cores):
        sc = 0  # slot-chunk cursor
        cc = 0  # column-chunk cursor
        for (j0, j1, widths) in tiles:
            wt = j1 - j0
            nodes = col_node[c, j0:j1]  # [wt], -1 for virtual
            real = nodes >= 0
            # column chunks (xcol gather + scatter + rdeg)
            nck = (wt + P - 1) // P
            for b in range(nck):
                a, e = b * P, min((b + 1) * P, wt)
                nb = nodes[a:e]
                rb = real[a:e]
                idx_col[c, : e - a, cc] = np.where(rb, nb, 0)
                idx_scat[c, : e - a, cc] = np.where(rb, nb % ncore_nodes, ncore_nodes)
                dg = np.where(rb, deg_all[np.where(rb, nb, 0)], 1).astype(np.float32)
                rdeg[c, :, cc * P : cc * P + (e - a)] = (1.0 / dg)[None, :]
                cc += 1
            # round slot chunks
            # per-column edge lists: r-th edge of node j
            for r, w in enumerate(widths):
                for b in range((w + P - 1) // P):
                    a, e = b * P, min((b + 1) * P, w)
                    nb = nodes[a:e]
                    rb = (nb >= 0) & (r < deg_all[np.where(nb >= 0, nb, 0)])
                    src = np.zeros(e - a, np.int64)
                    sel = np.where(rb)[0]
                    if len(sel):
                        eidx = order[starts[nb[sel]] + r]
                        src[sel] = row[eidx]
                    idx_row[c, : e - a, sc] = src
                    sc += 1
    return


def build_kernel(sched, n_nodes, in_ch, hid_ch, lat_ch, u_dim, n_graphs, ncore_nodes, variant=0):
    """Emit the Bass program (shared across cores)."""
    nc = bass.Bass(dynamic_dma_scratch_size=49152)
    if variant == 1:
        nc._fix_max_waits = 2
    if variant >= 3:
        nc._fix_dma_waits2 = True
    tiles = sched["tiles"]
    n_chunk_slots = sched["n_chunk_slots"]
    n_col_chunks = sched["n_col_chunks"]
    NB = ncore_nodes // P  # 6250/128 -> 48.8; use ceil
    n_ub = (ncore_nodes + P - 1) // P

    x_t = nc.dram_tensor("x", [n_nodes, in_ch], F32, kind="ExternalInput")
    u_t = nc.dram_tensor("u", [n_graphs, u_dim], F32, kind="ExternalInput")
    xown_t = nc.dram_tensor("x_own", [ncore_nodes, in_ch], F32, kind="ExternalInput")
    batch_t = nc.dram_tensor("batch_own", [P, n_ub], I32, kind="ExternalInput")
    w1a_t = nc.dram_tensor("W1a", [in_ch, hid_ch], F32, kind="ExternalInput")
    w1b_t = nc.dram_tensor("W1b", [in_ch, hid_ch], F32, kind="ExternalInput")
    w2_t = nc.dram_tensor("W2", [hid_ch, hid_ch], F32, kind="ExternalInput")
    w3_t = nc.dram_tensor("W3", [hid_ch, lat_ch], F32, kind="ExternalInput")
    b1_t = nc.dram_tensor("b1", [hid_ch, 1], F32, kind="ExternalInput")
    b2_t = nc.dram_tensor("b2", [hid_ch, 1], F32, kind="ExternalInput")
    b3_t = nc.dram_tensor("b3", [lat_ch, 1], F32, kind="ExternalInput")
    idxrow_t = nc.dram_tensor("idx_row", [P, n_chunk_slots], I32, kind="ExternalInput")
    idxcol_t = nc.dram_tensor("idx_col", [P, n_col_chunks], I32, kind="ExternalInput")
    idxscat_t = nc.dram_tensor("idx_scat", [P, n_col_chunks], I32, kind="ExternalInput")
    rdeg_t = nc.dram_tensor("rdeg", [64, n_col_chunks * P], F32, kind="ExternalInput")

    out_mid = nc.dram_tensor("out_mid", [ncore_nodes + 1, 3 * lat_ch], F32, kind="ExternalOutput")
    out_x = nc.dram_tensor("out_x", [ncore_nodes, in_ch], F32, kind="ExternalOutput")
    out_u = nc.dram_tensor("out_u", [ncore_nodes, u_dim], F32, kind="ExternalOutput")

    with tile.TileContext(nc) as tc:
        with (
            tc.tile_pool(name="const", bufs=1) as constp,
            tc.tile_pool(name="idxp", bufs=1) as idxp,
            tc.tile_pool(name="gat", bufs=(20 if variant >= 2 else 12)) as gatp,
            tc.tile_pool(name="act", bufs=4) as actp,
            tc.tile_pool(name="xcolp", bufs=2) as xcolp,
            tc.tile_pool(name="mm", bufs=2) as mmp,
            tc.tile_pool(name="stage", bufs=2) as stagep,
            tc.tile_pool(name="ps_h1", bufs=1, space="PSUM") as ps_h1,
            tc.tile_pool(name="ps_h3", bufs=2, space="PSUM") as ps_h3,
            tc.tile_pool(name="ps_sum", bufs=(2 if variant in (1, 2, 3) else 1), space="PSUM") as ps_sum,
            tc.tile_pool(name="ps_tr", bufs=2, space="PSUM") as ps_tr,
            tc.tile_pool(name="ps_fin", bufs=1, space="PSUM") as ps_fin,
        ):
            # constants
            w1a = constp.tile([in_ch, hid_ch], F32); nc.sync.dma_start(w1a[:], w1a_t[:])
            w1b = constp.tile([in_ch, hid_ch], F32); nc.sync.dma_start(w1b[:], w1b_t[:])
            w2 = constp.tile([hid_ch, hid_ch], F32); nc.sync.dma_start(w2[:], w2_t[:])
            w3 = constp.tile([hid_ch, lat_ch], F32); nc.sync.dma_start(w3[:], w3_t[:])
            b1 = constp.tile([hid_ch, 1], F32); nc.sync.dma_start(b1[:], b1_t[:])
            b2 = constp.tile([hid_ch, 1], F32); nc.sync.dma_start(b2[:], b2_t[:])
            b3 = constp.tile([lat_ch, 1], F32); nc.sync.dma_start(b3[:], b3_t[:])
            from concourse.masks import make_identity
            ident = constp.tile([P, P], F32)
            make_identity(nc, ident[:])
            ident64 = constp.tile([64, 64], F32)
            make_identity(nc, ident64[:])
            idx_row_sb = idxp.tile([P, n_chunk_slots], I32)
            nc.sync.dma_start(idx_row_sb[:], idxrow_t[:])
            idx_col_sb = idxp.tile([P, n_col_chunks], I32)
            nc.sync.dma_start(idx_col_sb[:], idxcol_t[:])
            idx_scat_sb = idxp.tile([P, n_col_chunks], I32)
            nc.sync.dma_start(idx_scat_sb[:], idxscat_t[:])
            rdeg_sb = idxp.tile([64, n_col_chunks * P], F32)
            nc.sync.dma_start(rdeg_sb[:], rdeg_t[:])

            # ---- side stream: x copy + u gather ----
            def side_stream():
                nc.sync.dma_start(out_x[:], xown_t[:])
                batch_sb = idxp.tile([P, n_ub], I32)
                nc.sync.dma_start(batch_sb[:], batch_t[:])
                for b in range(n_ub):
                    nrows = min(P, ncore_nodes - b * P)
                    ug = gatp.tile([P, u_dim], F32, tag="ug")
                    nc.gpsimd.indirect_dma_start(
                        out=ug[:], out_offset=None, in_=u_t[:],
                        in_offset=IndirectOffsetOnAxis(ap=batch_sb[:, b : b + 1], axis=0))
                    nc.sync.dma_start(out_u[b * P : b * P + nrows, :], ug[:nrows, :])
            if variant in (0, 6):
                side_stream()

            # ---- main loop ----
            sc = 0
            cc = 0
            for (j0, j1, widths) in tiles:
                wt = j1 - j0
                nck = (wt + P - 1) // P
                cc0 = cc
                # xcol gather + transpose -> xcolT [64, wt]
                xcolT = xcolp.tile([in_ch, W], F32, tag="xcolT")
                for b in range(nck):
                    g = gatp.tile([P, in_ch], F32, tag="gcol")
                    nc.gpsimd.indirect_dma_start(
                        out=g[:], out_offset=None, in_=x_t[:],
                        in_offset=IndirectOffsetOnAxis(ap=idx_col_sb[:, cc : cc + 1], axis=0))
                    ptr = ps_tr.tile([in_ch, W], F32, tag="ptr")
                    nc.tensor.transpose(out=ptr[:, b * P : (b + 1) * P], in_=g[:], identity=ident[:])
                    nc.scalar.activation(xcolT[:, b * P : (b + 1) * P], ptr[:, b * P : (b + 1) * P], AF.Copy)
                    cc += 1

                psum = ps_sum.tile([lat_ch, W], F32, tag="psum")
                vmax = actp.tile([lat_ch, W], F32, tag="vmax")
                vmin = actp.tile([lat_ch, W], F32, tag="vmin")

                d_t = len(widths)

                def stageA(w):
                    nonlocal sc
                    nrk = (w + P - 1) // P
                    xrowT = mmp.tile([in_ch, W], F32, tag="xrowT")
                    for b in range(nrk):
                        g = gatp.tile([P, in_ch], F32, tag="grow")
                        nc.gpsimd.indirect_dma_start(
                            out=g[:], out_offset=None, in_=x_t[:],
                            in_offset=IndirectOffsetOnAxis(ap=idx_row_sb[:, sc : sc + 1], axis=0))
                        ptr = ps_tr.tile([in_ch, W], F32, tag="ptr")
                        nc.tensor.transpose(out=ptr[:, b * P : (b + 1) * P], in_=g[:], identity=ident[:])
                        nc.scalar.activation(xrowT[:, b * P : (b + 1) * P], ptr[:, b * P : (b + 1) * P], AF.Copy)
                        sc += 1
                    h1p = ps_h1.tile([hid_ch, W], F32, tag="h1p")
                    nc.tensor.matmul(out=h1p[:, :w], lhsT=w1a[:], rhs=xrowT[:, :w], start=True, stop=False)
                    nc.tensor.matmul(out=h1p[:, :w], lhsT=w1b[:], rhs=xcolT[:, :w], start=False, stop=True)
                    h1 = actp.tile([hid_ch, W], F32, tag="h1")
                    nc.scalar.activation(h1[:, :w], h1p[:, :w], AF.Relu, bias=b1[:])
                    return h1

                def stageB(r, w, h1):
                    h2p = ps_h1.tile([hid_ch, W], F32, tag="h2p")
                    nc.tensor.matmul(out=h2p[:, :w], lhsT=w2[:], rhs=h1[:, :w], start=True, stop=True)
                    h2 = actp.tile([hid_ch, W], F32, tag="h2")
                    nc.scalar.activation(h2[:, :w], h2p[:, :w], AF.Relu, bias=b2[:])
                    h3p = ps_h3.tile([lat_ch, W], F32, tag="h3p")
                    nc.tensor.matmul(out=h3p[:, :w], lhsT=w3[:], rhs=h2[:, :w], start=True, stop=True)
                    nc.tensor.matmul(out=psum[:, :w], lhsT=w3[:], rhs=h2[:, :w],
                                     start=(r == 0), stop=(r == d_t - 1), skip_group_check=True)
                    if r == 0:
                        nc.vector.tensor_copy(vmax[:, :w], h3p[:, :w])
                        nc.vector.tensor_copy(vmin[:, :w], h3p[:, :w])
                    else:
                        nc.vector.tensor_tensor(out=vmax[:, :w], in0=vmax[:, :w], in1=h3p[:, :w], op=ALU.max)
                        nc.vector.tensor_tensor(out=vmin[:, :w], in0=vmin[:, :w], in1=h3p[:, :w], op=ALU.min)

                if variant >= 6:
                    h1_prev = stageA(widths[0])
                    for r in range(1, d_t):
                        h1_cur = stageA(widths[r])
                        stageB(r - 1, widths[r - 1], h1_prev)
                        h1_prev = h1_cur
                    stageB(d_t - 1, widths[d_t - 1], h1_prev)
                else:
                    for r, w in enumerate(widths):
                        stageB(r, w, stageA(w))

                # finalize tile: mean/max/min + b3, transpose to node-major, scatter
                mean_s = stagep.tile([lat_ch, W], F32, tag="mean_s")
                nc.vector.tensor_tensor(out=mean_s[:, :wt], in0=psum[:, :wt],
                                        in1=rdeg_sb[:, cc0 * P : cc0 * P + wt], op=ALU.mult)
                mean_f = stagep.tile([lat_ch, W], F32, tag="mean_f")
                nc.scalar.activation(mean_f[:, :wt], mean_s[:, :wt], AF.Identity, bias=b3[:])
                max_f = stagep.tile([lat_ch, W], F32, tag="max_f")
                nc.scalar.activation(max_f[:, :wt], vmax[:, :wt], AF.Identity, bias=b3[:])
                min_f = stagep.tile([lat_ch, W], F32, tag="min_f")
                nc.scalar.activation(min_f[:, :wt], vmin[:, :wt], AF.Identity, bias=b3[:])

                for b in range(nck):
                    e = min((b + 1) * P, wt) - b * P
                    stg = stagep.tile([P, 3 * lat_ch], F32, tag="stg")
                    for fi, f in enumerate((mean_f, max_f, min_f)):
                        pf = (ps_tr if variant in (1, 2, 3) else ps_fin).tile([P, lat_ch], F32, tag=("ptr" if variant in (1, 2, 3) else "pf"))
                        nc.tensor.transpose(out=pf[:], in_=f[:, b * P : (b + 1) * P], identity=ident64[:])
                        nc.scalar.activation(stg[:, fi * lat_ch : (fi + 1) * lat_ch], pf[:], AF.Copy)
                    nc.gpsimd.indirect_dma_start(
                        out=out_mid[:], out_offset=IndirectOffsetOnAxis(
                            ap=idx_scat_sb[:, cc0 + b : cc0 + b + 1], axis=0),
                        in_=stg[:], in_offset=None)
            if variant in (1, 2, 3, 5):
                side_stream()
    return nc


# ---------------- public entry point ----------------

N_NODES = 50000
N_EDGES = 800000
IN_CH = 64
HID_CH = 128
LAT_CH = 64
N_GRAPHS = 64
U_DIM = 32
N_CORES = 8


def kernel(**inputs):
    """Full-input NodeModel forward. Returns [N_NODES, 288] float32."""
    from concourse.bass_utils import run_bass_kernel_spmd

    x = np.asarray(inputs["x"], np.float32)
    edge_index = np.asarray(inputs["edge_index"])
    u = np.asarray(inputs["u"], np.float32)
    batch = np.asarray(inputs["batch"])
    W1 = np.asarray(inputs["W1"], np.float32)
    b1 = np.asarray(inputs["b1"], np.float32)
    W2 = np.asarray(inputs["W2"], np.float32)
    b2 = np.asarray(inputs["b2"], np.float32)
    W3 = np.asarray(inputs["W3"], np.float32)
    b3 = np.asarray(inputs["b3"], np.float32)

    n_nodes, in_ch = x.shape
    hid_ch = W2.shape[0]
    lat_ch = W3.shape[1]
    n_graphs, u_dim = u.shape

    row = edge_index[0].astype(np.int32)
    col = edge_index[1].astype(np.int32)

    sched = build_schedule(col, n_nodes, N_CORES, in_ch, lat_ch)
    fill_indices(sched, row, col, N_CORES)
    ncn = sched["ncore_nodes"]

    nc = build_kernel(sched, n_nodes, in_ch, hid_ch, lat_ch, u_dim, n_graphs, ncn)

    n_ub = (ncn + 127) // 128
    in_maps = []
    for c in range(N_CORES):
        bo = np.zeros((128, n_ub), np.int32)
        bvals = batch[c * ncn : (c + 1) * ncn].astype(np.int32)
        for b in range(n_ub):
            seg = bvals[b * 128 : (b + 1) * 128]
            bo[: len(seg), b] = seg
        in_maps.append({
            "x": x, "u": u, "x_own": np.ascontiguousarray(x[c * ncn : (c + 1) * ncn]),
            "batch_own": bo,
            "W1a": np.ascontiguousarray(W1[:in_ch]), "W1b": np.ascontiguousarray(W1[in_ch:]),
            "W2": W2, "W3": W3,
            "b1": np.ascontiguousarray(b1[:, None]), "b2": np.ascontiguousarray(b2[:, None]),
            "b3": np.ascontiguousarray(b3[:, None]),
            "idx_row": sched["idx_row"][c], "idx_col": sched["idx_col"][c],
            "idx_scat": sched["idx_scat"][c], "rdeg": sched["rdeg"][c],
        })

    res = run_bass_kernel_spmd(nc, in_maps, core_ids=list(range(N_CORES)))

    parts = []
    for c in range(N_CORES):
        r = res.results[c]
        mid = r["out_mid"][:ncn]
        parts.append(np.concatenate([r["out_x"], mid, r["out_u"]], axis=1))
    return np.concatenate(parts, axis=0).astype(np.float32)



# revision 3
# speedup vs baseline: 2.8585x; 2.8585x over previous
"""Patch TileContext._drain_and_barrier: this container's walrus codegen
rejects >2 sem waits on one CTRL (Drain) instruction. Split the kernel-tail
drain's waits across separate nop instructions (1 wait each)."""
import concourse.tile as tile  # noqa
import concourse.mybir as mybir
from concourse.vector_clock import ScopedClock
from concourse._compat import not_none as nn


def _drain_and_barrier_split(self, tick_clock, wait_clock):
    nc = self.nc
    carrier = nc.sync.nop()
    wait_clock.add_sem_waits(carrier.ins, ScopedClock({None: tick_clock.global_clock}))
    si = carrier.ins.sync_info
    waits = list(si.on_wait) if si and si.on_wait else []
    if len(waits) > 1:
        si.on_wait.clear()
        si.on_wait.append(waits[0])
        for w in waits[1:]:
            n2 = nc.sync.nop()
            n2.ins.sync_info = mybir.SyncInfo(on_wait=[w], on_update=[])
    nc.sync.drain()

    nc.all_engine_barrier()
    assert self.sems is not None
    popped = nc._tile_sem_poison_stack.pop()
    assert popped is self._sem_poison
    nc.clear_and_free_semaphores(list(self.sems.allocated().values()))
    nc.all_engine_barrier()


tile.TileContext._drain_and_barrier = _drain_and_barrier_split


# ---- global wait-splitting pass ----
# This walrus build packs at most MAX_WAITS sem-waits per instruction
# (ISA EVENTS struct holds one; codegen can prepend a limited number of
# sync-wait commands). Move excess waits onto InstNoOp carriers.
MAX_WAITS = 2

def fix_waits(nc, max_waits=MAX_WAITS):
    import concourse.mybir as mybir
    dma2 = getattr(nc, "_fix_dma_waits2", False)
    n_fixed = 0
    for fn in nc.m.functions:
        for blk in fn.blocks:
            insts = blk.instructions
            out = []
            for inst in insts:
                lim = max_waits
                if dma2 and isinstance(inst, mybir.InstDMACopy):
                    lim = 2
                si = getattr(inst, "sync_info", None)
                if si is not None and si.on_wait and len(si.on_wait) > lim:
                    waits = list(si.on_wait)
                    si.on_wait.clear()
                    for w in waits[:-lim] if lim else waits:
                        n_fixed += 1
                        nop = mybir.InstNoOp(
                            name=f"{inst.name}.wsplit{n_fixed}",
                            sync_info=mybir.SyncInfo(on_wait=[w], on_update=[]),
                            bass_nofuse=True,
                            engine=inst.engine,
                        )
                        out.append(nop)
                    for w in waits[-lim:] if lim else []:
                        si.on_wait.append(w)
                elif si is not None and si.on_wait and len(si.on_wait) > 1 and getattr(inst, "opcode", None) is None:
                    pass
                out.append(inst)
            blk.instructions = out
    return n_fixed


# auto-apply fix_waits on serialization
import concourse.bass as _bass
_orig_to_json_bytes = _bass.Bass.to_json_bytes

def _to_json_bytes_fixed(self, *a, **kw):
    try:
        fix_waits(self, max_waits=getattr(self, "_fix_max_waits", 1))
    except Exception as e:
        import traceback; traceback.print_exc()
    return _orig_to_json_bytes(self, *a, **kw)

_bass.Bass.to_json_bytes = _to_json_bytes_fixed


"""NodeModel GNN kernel for Trainium2 (Bass/Tile), 8-core SPMD. v2.

Strategy (v2 — fp16 compute, lean device program):
- Shard destination NODES into 8 contiguous ranges of 6250; each core handles
  exactly the edges targeting its nodes (no collectives needed).
- Shared degree-sorted column schedule (same as v1): columns = destination
  nodes grouped by degree descending; round r feeds the r-th edge of each
  still-active column. Segment SUM accumulates in PSUM across rounds via
  matmul; MAX/MIN are running DVE ops; COUNT is host-known (rdeg).
- fp16 everywhere on the device data path (x gathered in fp16, weights fp16,
  activations fp16); PSUM accumulation stays fp32.
- Column-side x (xcolT) and 1/deg are host-prepared per core in schedule
  order, loaded by direct DMA — no column gathers or transposes on device.
- Output is written column-major ([192, cols]) by direct DMA; the host
  un-permutes columns to node order and assembles the final concat (x and
  u[batch] passthrough fields are host-assembled).
- Only per-edge source-row gathers use indirect DMA (128 rows/instruction,
  the Pool-engine SWDGE serial floor dominates the kernel).
"""

import numpy as np

import concourse.bass as bass
import concourse.tile as tile
from concourse.bass import IndirectOffsetOnAxis

F32 = mybir.dt.float32
F16 = mybir.dt.float16
I32 = mybir.dt.int32
AF = mybir.ActivationFunctionType
ALU = mybir.AluOpType

P = 128
W = 512  # tile width (columns = destination nodes)


def build_schedule(col, n_nodes, n_cores):
    """Host-side index preprocessing. Returns shared schedule + per-core arrays."""
    ncore_nodes = n_nodes // n_cores
    deg_all = np.bincount(col, minlength=n_nodes)
    dmax = int(deg_all.max())

    # per-core degree histograms of own nodes
    hist = np.zeros((n_cores, dmax + 1), np.int64)
    for c in range(n_cores):
        d = deg_all[c * ncore_nodes : (c + 1) * ncore_nodes]
        hist[c] = np.bincount(d, minlength=dmax + 1)
    H = hist.max(axis=0)  # shared histogram (per exact degree), index 0 unused

    # shared column degree sequence, descending
    col_degs = np.repeat(np.arange(dmax, 0, -1), H[dmax:0:-1])
    n_cols = len(col_degs)
    n_tiles = (n_cols + W - 1) // W

    # CSR of edges by destination (stable order)
    order = np.argsort(col, kind="stable")
    starts = np.zeros(n_nodes + 1, np.int64)
    np.cumsum(deg_all, out=starts[1:])

    # per-core: map shared columns -> node ids (real) or -1 (virtual)
    col_node = np.full((n_cores, n_cols), -1, np.int64)
    for c in range(n_cores):
        d_own = deg_all[c * ncore_nodes : (c + 1) * ncore_nodes]
        nodes_by_deg = {}
        for ln in np.argsort(-d_own, kind="stable"):
            if d_own[ln] == 0:
                break
            nodes_by_deg.setdefault(int(d_own[ln]), []).append(ln)
        used = {d: 0 for d in range(1, dmax + 1)}
        for j in range(n_cols):
            d = int(col_degs[j])
            lst = nodes_by_deg.get(d, [])
            k = used[d]
            if k < len(lst):
                col_node[c, j] = c * ncore_nodes + lst[k]
                used[d] = k + 1

    # schedule: per tile, list of round widths; global column -> padded pos
    tiles = []
    col_pos = np.zeros(n_cols, np.int64)
    cc = 0
    for t in range(n_tiles):
        j0, j1 = t * W, min((t + 1) * W, n_cols)
        degs = col_degs[j0:j1]
        d_t = int(degs[0])
        widths = [int(np.searchsorted(-degs, -(r + 1), side="right")) for r in range(d_t)]
        tiles.append((j0, j1, widths, cc))
        col_pos[j0:j1] = cc * P + np.arange(j1 - j0)
        cc += (j1 - j0 + P - 1) // P

    n_chunk_slots = sum(sum((w + P - 1) // P for w in widths) for _, _, widths, _ in tiles)
    n_col_chunks = cc

    return dict(
        ncore_nodes=ncore_nodes, deg_all=deg_all, col_degs=col_degs,
        n_cols=n_cols, n_tiles=n_tiles, tiles=tiles, order=order, starts=starts,
        col_node=col_node, col_pos=col_pos,
        n_chunk_slots=n_chunk_slots, n_col_chunks=n_col_chunks, dmax=dmax,
    )


def fill_row_indices(sched, row, n_cores):
    """Per-core gather index array idx_row [P, n_chunk_slots] (source of the
    r-th edge of each column; 0 for virtual/padding lanes)."""
    order, starts = sched["order"], sched["starts"]
    col_node = sched["col_node"]
    tiles = sched["tiles"]
    deg_all = sched["deg_all"]
    idx_row = np.zeros((n_cores, P, sched["n_chunk_slots"]), np.int32)

    for c in range(n_cores):
        sc = 0
        for (j0, j1, widths, _cc0) in tiles:
            nodes = col_node[c, j0:j1]
            for r, w in enumerate(widths):
                for b in range((w + P - 1) // P):
                    a, e = b * P, min((b + 1) * P, w)
                    nb = nodes[a:e]
                    rb = (nb >= 0) & (r < deg_all[np.where(nb >= 0, nb, 0)])
                    src = np.zeros(e - a, np.int64)
                    sel = np.where(rb)[0]
                    if len(sel):
                        eidx = order[starts[nb[sel]] + r]
                        src[sel] = row[eidx]
                    idx_row[c, : e - a, sc] = src
                    sc += 1
    return idx_row


def build_kernel(sched, n_nodes, hid_ch, lat_ch):
    """Emit the Bass program (shared across cores)."""
    nc = bass.Bass(dynamic_dma_scratch_size=49152)
    tiles = sched["tiles"]
    n_chunk_slots = sched["n_chunk_slots"]
    ncc = sched["n_col_chunks"]

    x16_t = nc.dram_tensor("x16", [n_nodes, 64], F16, kind="ExternalInput")
    idxrow_t = nc.dram_tensor("idx_row", [P, n_chunk_slots], I32, kind="ExternalInput")
    xcolT_t = nc.dram_tensor("xcolT", [64, ncc * P], F16, kind="ExternalInput")
    rdeg_t = nc.dram_tensor("rdeg", [64, ncc * P], F32, kind="ExternalInput")
    w1a_t = nc.dram_tensor("W1a", [64, hid_ch], F16, kind="ExternalInput")
    w1b_t = nc.dram_tensor("W1b", [64, hid_ch], F16, kind="ExternalInput")
    w2_t = nc.dram_tensor("W2", [hid_ch, hid_ch], F16, kind="ExternalInput")
    w3_t = nc.dram_tensor("W3", [hid_ch, lat_ch], F16, kind="ExternalInput")
    b1_t = nc.dram_tensor("b1", [hid_ch, 1], F32, kind="ExternalInput")
    b2_t = nc.dram_tensor("b2", [hid_ch, 1], F32, kind="ExternalInput")
    b3_t = nc.dram_tensor("b3", [lat_ch, 1], F32, kind="ExternalInput")
    ident_t = nc.dram_tensor("ident", [P, P], F16, kind="ExternalInput")

    outT_t = nc.dram_tensor("outT", [3 * lat_ch, ncc * P], F32, kind="ExternalOutput")

    with tile.TileContext(nc) as tc:
        with (
            tc.tile_pool(name="const", bufs=1) as constp,
            tc.tile_pool(name="idxp", bufs=1) as idxp,
            tc.tile_pool(name="gat", bufs=12) as gatp,
            tc.tile_pool(name="xrow", bufs=2) as xrowp,
            tc.tile_pool(name="xcol", bufs=2) as xcolp,
            tc.tile_pool(name="act", bufs=4) as actp,
            tc.tile_pool(name="mm", bufs=4) as mmp,
            tc.tile_pool(name="stage", bufs=8) as stagep,
            tc.tile_pool(name="ps_tr", bufs=2, space="PSUM") as ps_tr,
            tc.tile_pool(name="ps_h", bufs=1, space="PSUM") as ps_h,
            tc.tile_pool(name="ps_h3", bufs=2, space="PSUM") as ps_h3,
            tc.tile_pool(name="ps_sum", bufs=1, space="PSUM") as ps_sum,
        ):
            # constants
            w1a = constp.tile([64, hid_ch], F16); nc.sync.dma_start(w1a[:], w1a_t[:])
            w1b = constp.tile([64, hid_ch], F16); nc.sync.dma_start(w1b[:], w1b_t[:])
            w2 = constp.tile([hid_ch, hid_ch], F16); nc.sync.dma_start(w2[:], w2_t[:])
            w3 = constp.tile([hid_ch, lat_ch], F16); nc.sync.dma_start(w3[:], w3_t[:])
            b1 = constp.tile([hid_ch, 1], F32); nc.sync.dma_start(b1[:], b1_t[:])
            b2 = constp.tile([hid_ch, 1], F32); nc.sync.dma_start(b2[:], b2_t[:])
            b3 = constp.tile([lat_ch, 1], F32); nc.sync.dma_start(b3[:], b3_t[:])
            ident = constp.tile([P, P], F16); nc.sync.dma_start(ident[:], ident_t[:])
            idx_row_sb = idxp.tile([P, n_chunk_slots], I32)
            nc.sync.dma_start(idx_row_sb[:], idxrow_t[:])
            rdeg_sb = idxp.tile([64, ncc * P], F32)
            nc.sync.dma_start(rdeg_sb[:], rdeg_t[:])

            sc = 0
            for (j0, j1, widths, cc0) in tiles:
                wt = j1 - j0
                d_t = len(widths)

                xcolT = xcolp.tile([64, W], F16, tag="xcolT")
                nc.sync.dma_start(xcolT[:, :wt], xcolT_t[:, cc0 * P : cc0 * P + wt])

                psum = ps_sum.tile([lat_ch, W], F32, tag="psum")
                vmax = mmp.tile([lat_ch, W], F16, tag="vmax")
                vmin = mmp.tile([lat_ch, W], F16, tag="vmin")

                for r, w in enumerate(widths):
                    nrk = (w + P - 1) // P
                    gs = []
                    for b in range(nrk):
                        g = gatp.tile([P, 64], F16, tag="g")
                        nc.gpsimd.indirect_dma_start(
                            out=g[:], out_offset=None, in_=x16_t[:],
                            in_offset=IndirectOffsetOnAxis(ap=idx_row_sb[:, sc : sc + 1], axis=0))
                        gs.append(g)
                        sc += 1
                    ptr = ps_tr.tile([64, W], F16, tag="ptr")
                    for b in range(nrk):
                        nc.tensor.transpose(out=ptr[:, b * P : (b + 1) * P], in_=gs[b][:], identity=ident[:])
                    xrowT = xrowp.tile([64, W], F16, tag="xrowT")
                    nc.vector.tensor_copy(xrowT[:, : nrk * P], ptr[:, : nrk * P])

                    h1p = ps_h.tile([hid_ch, W], F32, tag="h1p")
                    nc.tensor.matmul(out=h1p[:, :w], lhsT=w1a[:], rhs=xrowT[:, :w], start=True, stop=False)
                    nc.tensor.matmul(out=h1p[:, :w], lhsT=w1b[:], rhs=xcolT[:, :w], start=False, stop=True)
                    h1 = actp.tile([hid_ch, W], F16, tag="h1")
                    nc.scalar.activation(h1[:, :w], h1p[:, :w], AF.Relu, bias=b1[:])
                    h2p = ps_h.tile([hid_ch, W], F32, tag="h2p")
                    nc.tensor.matmul(out=h2p[:, :w], lhsT=w2[:], rhs=h1[:, :w], start=True, stop=True)
                    h2 = actp.tile([hid_ch, W], F16, tag="h2")
                    nc.scalar.activation(h2[:, :w], h2p[:, :w], AF.Relu, bias=b2[:])
                    h3p = ps_h3.tile([lat_ch, W], F32, tag="h3p")
                    nc.tensor.matmul(out=h3p[:, :w], lhsT=w3[:], rhs=h2[:, :w], start=True, stop=True)
                    nc.tensor.matmul(out=psum[:, :w], lhsT=w3[:], rhs=h2[:, :w],
                                     start=(r == 0), stop=(r == d_t - 1), skip_group_check=True)
                    if r == 0:
                        nc.vector.tensor_copy(vmax[:, :w], h3p[:, :w])
                        nc.vector.tensor_copy(vmin[:, :w], h3p[:, :w])
                    else:
                        nc.vector.tensor_tensor(out=vmax[:, :w], in0=vmax[:, :w], in1=h3p[:, :w], op=ALU.max)
                        nc.vector.tensor_tensor(out=vmin[:, :w], in0=vmin[:, :w], in1=h3p[:, :w], op=ALU.min)

                # finalize tile: mean/max/min + b3, direct column-major writes
                mean_s = stagep.tile([lat_ch, W], F32, tag="mean_s")
                nc.vector.tensor_tensor(out=mean_s[:, :wt], in0=psum[:, :wt],
                                        in1=rdeg_sb[:, cc0 * P : cc0 * P + wt], op=ALU.mult)
                mean_f = stagep.tile([lat_ch, W], F32, tag="mean_f")
                nc.scalar.activation(mean_f[:, :wt], mean_s[:, :wt], AF.Identity, bias=b3[:])
                max_f = stagep.tile([lat_ch, W], F32, tag="max_f")
                nc.scalar.activation(max_f[:, :wt], vmax[:, :wt], AF.Identity, bias=b3[:])
                min_f = stagep.tile([lat_ch, W], F32, tag="min_f")
                nc.scalar.activation(min_f[:, :wt], vmin[:, :wt], AF.Identity, bias=b3[:])
                nc.sync.dma_start(outT_t[0:lat_ch, cc0 * P : cc0 * P + wt], mean_f[:, :wt])
                nc.sync.dma_start(outT_t[lat_ch : 2 * lat_ch, cc0 * P : cc0 * P + wt], max_f[:, :wt])
                nc.sync.dma_start(outT_t[2 * lat_ch : 3 * lat_ch, cc0 * P : cc0 * P + wt], min_f[:, :wt])
    return nc


# ---------------- public entry point ----------------

N_NODES = 50000
N_EDGES = 800000
IN_CH = 64
HID_CH = 128
LAT_CH = 64
N_GRAPHS = 64
U_DIM = 32
N_CORES = 8


def make_in_maps(sched, x, W1, W2, W3, b1, b2, b3):
    """Per-core input dicts (shared program, per-core data)."""
    x16 = x.astype(np.float16)
    ncc = sched["n_col_chunks"]
    col_node = sched["col_node"]
    col_pos = sched["col_pos"]
    deg_all = sched["deg_all"]
    n_cols = sched["n_cols"]
    ident = np.eye(P, dtype=np.float16)

    in_maps = []
    for c in range(N_CORES):
        nodes = col_node[c]  # [n_cols], -1 virtual
        real = nodes >= 0
        xcolT = np.zeros((64, ncc * P), np.float16)
        xcolT[:, col_pos[real]] = x16[nodes[real]].T
        rdeg = np.ones((1, ncc * P), np.float32)
        rdeg[0, col_pos[real]] = 1.0 / deg_all[nodes[real]]
        rdeg = np.broadcast_to(rdeg, (64, ncc * P)).copy()
        in_maps.append({
            "x16": x16,
            "idx_row": sched["idx_row"][c],
            "xcolT": xcolT, "rdeg": rdeg,
            "W1a": W1[:64].astype(np.float16), "W1b": W1[64:].astype(np.float16),
            "W2": W2.astype(np.float16), "W3": W3.astype(np.float16),
            "b1": np.ascontiguousarray(b1[:, None].astype(np.float32)),
            "b2": np.ascontiguousarray(b2[:, None].astype(np.float32)),
            "b3": np.ascontiguousarray(b3[:, None].astype(np.float32)),
            "ident": ident,
        })
    return in_maps


def assemble_output(sched, res_list, x, u, batch):
    """Un-permute per-core column-major results and build the full output."""
    n_nodes = x.shape[0]
    col_node = sched["col_node"]
    col_pos = sched["col_pos"]
    out = np.zeros((n_nodes, 288), np.float32)
    out[:, 0:64] = x
    out[:, 256:288] = u[batch]
    for c in range(N_CORES):
        outT = res_list[c]["outT"]  # [192, ncc*P]
        nodes = col_node[c]
        real = nodes >= 0
        out[nodes[real], 64:256] = outT[:, col_pos[real]].T
    return out


def kernel(**inputs):
    """Full-input NodeModel forward. Returns [N_NODES, 288] float32."""
    from concourse.bass_utils import run_bass_kernel_spmd

    x = np.asarray(inputs["x"], np.float32)
    edge_index = np.asarray(inputs["edge_index"])
    u = np.asarray(inputs["u"], np.float32)
    batch = np.asarray(inputs["batch"])
    W1 = np.asarray(inputs["W1"], np.float32)
    b1 = np.asarray(inputs["b1"], np.float32)
    W2 = np.asarray(inputs["W2"], np.float32)
    b2 = np.asarray(inputs["b2"], np.float32)
    W3 = np.asarray(inputs["W3"], np.float32)
    b3 = np.asarray(inputs["b3"], np.float32)

    row = edge_index[0].astype(np.int32)
    col = edge_index[1].astype(np.int32)

    sched = build_schedule(col, x.shape[0], N_CORES)
    sched["idx_row"] = fill_row_indices(sched, row, N_CORES)

    nc = build_kernel(sched, x.shape[0], W2.shape[0], W3.shape[1])
    in_maps = make_in_maps(sched, x, W1, W2, W3, b1, b2, b3)

    res = run_bass_kernel_spmd(nc, in_maps, core_ids=list(range(N_CORES)))
    return assemble_output(sched, res.results, x, u, batch).astype(np.float32)


# revision 6
# speedup vs baseline: 46.2170x; 16.1682x over previous
"""Patch TileContext._drain_and_barrier: this container's walrus codegen
rejects >2 sem waits on one CTRL (Drain) instruction. Split the kernel-tail
drain's waits across separate nop instructions (1 wait each)."""
import concourse.tile as tile  # noqa
import concourse.mybir as mybir
from concourse.vector_clock import ScopedClock
from concourse._compat import not_none as nn


def _drain_and_barrier_split(self, tick_clock, wait_clock):
    nc = self.nc
    carrier = nc.sync.nop()
    wait_clock.add_sem_waits(carrier.ins, ScopedClock({None: tick_clock.global_clock}))
    si = carrier.ins.sync_info
    waits = list(si.on_wait) if si and si.on_wait else []
    if len(waits) > 1:
        si.on_wait.clear()
        si.on_wait.append(waits[0])
        for w in waits[1:]:
            n2 = nc.sync.nop()
            n2.ins.sync_info = mybir.SyncInfo(on_wait=[w], on_update=[])
    nc.sync.drain()

    nc.all_engine_barrier()
    assert self.sems is not None
    popped = nc._tile_sem_poison_stack.pop()
    assert popped is self._sem_poison
    nc.clear_and_free_semaphores(list(self.sems.allocated().values()))
    nc.all_engine_barrier()


tile.TileContext._drain_and_barrier = _drain_and_barrier_split


# ---- global wait-splitting pass ----
# This walrus build packs at most MAX_WAITS sem-waits per instruction
# (ISA EVENTS struct holds one; codegen can prepend a limited number of
# sync-wait commands). Move excess waits onto InstNoOp carriers.
MAX_WAITS = 2

def fix_waits(nc, max_waits=MAX_WAITS):
    import concourse.mybir as mybir
    dma2 = getattr(nc, "_fix_dma_waits2", False)
    n_fixed = 0
    for fn in nc.m.functions:
        for blk in fn.blocks:
            insts = blk.instructions
            out = []
            for inst in insts:
                lim = max_waits
                if dma2 and isinstance(inst, mybir.InstDMACopy):
                    lim = 2
                si = getattr(inst, "sync_info", None)
                if si is not None and si.on_wait and len(si.on_wait) > lim:
                    waits = list(si.on_wait)
                    si.on_wait.clear()
                    for w in waits[:-lim] if lim else waits:
                        n_fixed += 1
                        nop = mybir.InstNoOp(
                            name=f"{inst.name}.wsplit{n_fixed}",
                            sync_info=mybir.SyncInfo(on_wait=[w], on_update=[]),
                            bass_nofuse=True,
                            engine=inst.engine,
                        )
                        out.append(nop)
                    for w in waits[-lim:] if lim else []:
                        si.on_wait.append(w)
                elif si is not None and si.on_wait and len(si.on_wait) > 1 and getattr(inst, "opcode", None) is None:
                    pass
                out.append(inst)
            blk.instructions = out
    return n_fixed


# auto-apply fix_waits on serialization
import concourse.bass as _bass
_orig_to_json_bytes = _bass.Bass.to_json_bytes

def _to_json_bytes_fixed(self, *a, **kw):
    try:
        fix_waits(self, max_waits=getattr(self, "_fix_max_waits", 1))
    except Exception as e:
        import traceback; traceback.print_exc()
    return _orig_to_json_bytes(self, *a, **kw)

_bass.Bass.to_json_bytes = _to_json_bytes_fixed


"""NodeModel GNN kernel for Trainium2 (Bass/Tile), 8-core SPMD. v2.

Strategy (v2 — fp16 compute, lean device program):
- Shard destination NODES into 8 contiguous ranges of 6250; each core handles
  exactly the edges targeting its nodes (no collectives needed).
- Shared degree-sorted column schedule (same as v1): columns = destination
  nodes grouped by degree descending; round r feeds the r-th edge of each
  still-active column. Segment SUM accumulates in PSUM across rounds via
  matmul; MAX/MIN are running DVE ops; COUNT is host-known (rdeg).
- fp16 everywhere on the device data path (x gathered in fp16, weights fp16,
  activations fp16); PSUM accumulation stays fp32.
- Column-side x (xcolT) and 1/deg are host-prepared per core in schedule
  order, loaded by direct DMA — no column gathers or transposes on device.
- Output is written column-major ([192, cols]) by direct DMA; the host
  un-permutes columns to node order and assembles the final concat (x and
  u[batch] passthrough fields are host-assembled).
- Only per-edge source-row gathers use indirect DMA (128 rows/instruction,
  the Pool-engine SWDGE serial floor dominates the kernel).
"""

import numpy as np

import concourse.bass as bass
import concourse.tile as tile
from concourse.bass import IndirectOffsetOnAxis

F32 = mybir.dt.float32
F16 = mybir.dt.float16
I32 = mybir.dt.int32
AF = mybir.ActivationFunctionType
ALU = mybir.AluOpType

P = 128
W = 512  # tile width (columns = destination nodes)


def build_schedule(col, n_nodes, n_cores):
    """Host-side index preprocessing. Returns shared schedule + per-core arrays."""
    ncore_nodes = n_nodes // n_cores
    deg_all = np.bincount(col, minlength=n_nodes)
    dmax = int(deg_all.max())

    # per-core degree histograms of own nodes
    hist = np.zeros((n_cores, dmax + 1), np.int64)
    for c in range(n_cores):
        d = deg_all[c * ncore_nodes : (c + 1) * ncore_nodes]
        hist[c] = np.bincount(d, minlength=dmax + 1)
    H = hist.max(axis=0)  # shared histogram (per exact degree), index 0 unused

    # shared column degree sequence, descending
    col_degs = np.repeat(np.arange(dmax, 0, -1), H[dmax:0:-1])
    n_cols = len(col_degs)
    n_tiles = (n_cols + W - 1) // W

    # CSR of edges by destination (stable order)
    order = np.argsort(col, kind="stable")
    starts = np.zeros(n_nodes + 1, np.int64)
    np.cumsum(deg_all, out=starts[1:])

    # per-core: map shared columns -> node ids (real) or -1 (virtual)
    col_node = np.full((n_cores, n_cols), -1, np.int64)
    for c in range(n_cores):
        d_own = deg_all[c * ncore_nodes : (c + 1) * ncore_nodes]
        nodes_by_deg = {}
        for ln in np.argsort(-d_own, kind="stable"):
            if d_own[ln] == 0:
                break
            nodes_by_deg.setdefault(int(d_own[ln]), []).append(ln)
        used = {d: 0 for d in range(1, dmax + 1)}
        for j in range(n_cols):
            d = int(col_degs[j])
            lst = nodes_by_deg.get(d, [])
            k = used[d]
            if k < len(lst):
                col_node[c, j] = c * ncore_nodes + lst[k]
                used[d] = k + 1

    # schedule: per tile, list of round widths; global column -> padded pos
    tiles = []
    col_pos = np.zeros(n_cols, np.int64)
    cc = 0
    for t in range(n_tiles):
        j0, j1 = t * W, min((t + 1) * W, n_cols)
        degs = col_degs[j0:j1]
        d_t = int(degs[0])
        widths = [int(np.searchsorted(-degs, -(r + 1), side="right")) for r in range(d_t)]
        tiles.append((j0, j1, widths, cc))
        col_pos[j0:j1] = cc * P + np.arange(j1 - j0)
        cc += (j1 - j0 + P - 1) // P

    n_chunk_slots = sum(sum((w + P - 1) // P for w in widths) for _, _, widths, _ in tiles)
    n_col_chunks = cc

    return dict(
        ncore_nodes=ncore_nodes, deg_all=deg_all, col_degs=col_degs,
        n_cols=n_cols, n_tiles=n_tiles, tiles=tiles, order=order, starts=starts,
        col_node=col_node, col_pos=col_pos,
        n_chunk_slots=n_chunk_slots, n_col_chunks=n_col_chunks, dmax=dmax,
    )


def fill_row_indices(sched, row, n_cores):
    """Per-core gather index array idx_row [P, n_chunk_slots] (source of the
    r-th edge of each column; 0 for virtual/padding lanes)."""
    order, starts = sched["order"], sched["starts"]
    col_node = sched["col_node"]
    tiles = sched["tiles"]
    deg_all = sched["deg_all"]
    idx_row = np.zeros((n_cores, P, sched["n_chunk_slots"]), np.int32)

    for c in range(n_cores):
        sc = 0
        for (j0, j1, widths, _cc0) in tiles:
            nodes = col_node[c, j0:j1]
            for r, w in enumerate(widths):
                for b in range((w + P - 1) // P):
                    a, e = b * P, min((b + 1) * P, w)
                    nb = nodes[a:e]
                    rb = (nb >= 0) & (r < deg_all[np.where(nb >= 0, nb, 0)])
                    src = np.zeros(e - a, np.int64)
                    sel = np.where(rb)[0]
                    if len(sel):
                        eidx = order[starts[nb[sel]] + r]
                        src[sel] = row[eidx]
                    idx_row[c, : e - a, sc] = src
                    sc += 1
    return idx_row


def build_kernel(sched, n_nodes, hid_ch, lat_ch, repeat=1):
    """Emit the Bass program (shared across cores). repeat>1 re-runs the whole
    tile loop (for timing regression only; outputs are simply overwritten)."""
    nc = bass.Bass(dynamic_dma_scratch_size=49152)
    tiles = sched["tiles"]
    n_chunk_slots = sched["n_chunk_slots"]
    ncc = sched["n_col_chunks"]

    x16_t = nc.dram_tensor("x16", [n_nodes, 64], F16, kind="ExternalInput")
    idxrow_t = nc.dram_tensor("idx_row", [P, n_chunk_slots], I32, kind="ExternalInput")
    xcolT_t = nc.dram_tensor("xcolT", [64, ncc * P], F16, kind="ExternalInput")
    rdeg_t = nc.dram_tensor("rdeg", [64, ncc * P], F32, kind="ExternalInput")
    w1a_t = nc.dram_tensor("W1a", [64, hid_ch], F16, kind="ExternalInput")
    w1b_t = nc.dram_tensor("W1b", [64, hid_ch], F16, kind="ExternalInput")
    w2_t = nc.dram_tensor("W2", [hid_ch, hid_ch], F16, kind="ExternalInput")
    w3_t = nc.dram_tensor("W3", [hid_ch, lat_ch], F16, kind="ExternalInput")
    b1_t = nc.dram_tensor("b1", [hid_ch, 1], F32, kind="ExternalInput")
    b2_t = nc.dram_tensor("b2", [hid_ch, 1], F32, kind="ExternalInput")
    b3_t = nc.dram_tensor("b3", [lat_ch, 1], F32, kind="ExternalInput")
    ident_t = nc.dram_tensor("ident", [P, P], F16, kind="ExternalInput")

    outT_t = nc.dram_tensor("outT", [3 * lat_ch, ncc * P], F32, kind="ExternalOutput")

    with tile.TileContext(nc) as tc:
        with (
            tc.tile_pool(name="const", bufs=1) as constp,
            tc.tile_pool(name="idxp", bufs=1) as idxp,
            tc.tile_pool(name="gat", bufs=12) as gatp,
            tc.tile_pool(name="xrow", bufs=2) as xrowp,
            tc.tile_pool(name="xcol", bufs=2) as xcolp,
            tc.tile_pool(name="act", bufs=4) as actp,
            tc.tile_pool(name="mm", bufs=4) as mmp,
            tc.tile_pool(name="stage", bufs=8) as stagep,
            tc.tile_pool(name="ps_tr", bufs=2, space="PSUM") as ps_tr,
            tc.tile_pool(name="ps_h", bufs=1, space="PSUM") as ps_h,
            tc.tile_pool(name="ps_h3", bufs=2, space="PSUM") as ps_h3,
            tc.tile_pool(name="ps_sum", bufs=1, space="PSUM") as ps_sum,
        ):
            # constants
            w1a = constp.tile([64, hid_ch], F16); nc.sync.dma_start(w1a[:], w1a_t[:])
            w1b = constp.tile([64, hid_ch], F16); nc.sync.dma_start(w1b[:], w1b_t[:])
            w2 = constp.tile([hid_ch, hid_ch], F16); nc.sync.dma_start(w2[:], w2_t[:])
            w3 = constp.tile([hid_ch, lat_ch], F16); nc.sync.dma_start(w3[:], w3_t[:])
            b1 = constp.tile([hid_ch, 1], F32); nc.sync.dma_start(b1[:], b1_t[:])
            b2 = constp.tile([hid_ch, 1], F32); nc.sync.dma_start(b2[:], b2_t[:])
            b3 = constp.tile([lat_ch, 1], F32); nc.sync.dma_start(b3[:], b3_t[:])
            ident = constp.tile([P, P], F16); nc.sync.dma_start(ident[:], ident_t[:])
            idx_row_sb = idxp.tile([P, n_chunk_slots], I32)
            nc.sync.dma_start(idx_row_sb[:], idxrow_t[:])
            rdeg_sb = idxp.tile([64, ncc * P], F32)
            nc.sync.dma_start(rdeg_sb[:], rdeg_t[:])

            for _rep in range(repeat):
              sc = 0
              for (j0, j1, widths, cc0) in tiles:
                wt = j1 - j0
                d_t = len(widths)

                xcolT = xcolp.tile([64, W], F16, tag="xcolT")
                nc.sync.dma_start(xcolT[:, :wt], xcolT_t[:, cc0 * P : cc0 * P + wt])

                psum = ps_sum.tile([lat_ch, W], F32, tag="psum")
                vmax = mmp.tile([lat_ch, W], F16, tag="vmax")
                vmin = mmp.tile([lat_ch, W], F16, tag="vmin")

                for r, w in enumerate(widths):
                    nrk = (w + P - 1) // P
                    gs = []
                    for b in range(nrk):
                        g = gatp.tile([P, 64], F16, tag="g")
                        nc.gpsimd.indirect_dma_start(
                            out=g[:], out_offset=None, in_=x16_t[:],
                            in_offset=IndirectOffsetOnAxis(ap=idx_row_sb[:, sc : sc + 1], axis=0))
                        gs.append(g)
                        sc += 1
                    ptr = ps_tr.tile([64, W], F16, tag="ptr")
                    for b in range(nrk):
                        nc.tensor.transpose(out=ptr[:, b * P : (b + 1) * P], in_=gs[b][:], identity=ident[:])
                    xrowT = xrowp.tile([64, W], F16, tag="xrowT")
                    nc.vector.tensor_copy(xrowT[:, : nrk * P], ptr[:, : nrk * P])

                    h1p = ps_h.tile([hid_ch, W], F32, tag="h1p")
                    nc.tensor.matmul(out=h1p[:, :w], lhsT=w1a[:], rhs=xrowT[:, :w], start=True, stop=False)
                    nc.tensor.matmul(out=h1p[:, :w], lhsT=w1b[:], rhs=xcolT[:, :w], start=False, stop=True)
                    h1 = actp.tile([hid_ch, W], F16, tag="h1")
                    nc.scalar.activation(h1[:, :w], h1p[:, :w], AF.Relu, bias=b1[:])
                    h2p = ps_h.tile([hid_ch, W], F32, tag="h2p")
                    nc.tensor.matmul(out=h2p[:, :w], lhsT=w2[:], rhs=h1[:, :w], start=True, stop=True)
                    h2 = actp.tile([hid_ch, W], F16, tag="h2")
                    nc.scalar.activation(h2[:, :w], h2p[:, :w], AF.Relu, bias=b2[:])
                    h3p = ps_h3.tile([lat_ch, W], F32, tag="h3p")
                    nc.tensor.matmul(out=h3p[:, :w], lhsT=w3[:], rhs=h2[:, :w], start=True, stop=True)
                    nc.tensor.matmul(out=psum[:, :w], lhsT=w3[:], rhs=h2[:, :w],
                                     start=(r == 0), stop=(r == d_t - 1), skip_group_check=True)
                    if r == 0:
                        nc.vector.tensor_copy(vmax[:, :w], h3p[:, :w])
                        nc.vector.tensor_copy(vmin[:, :w], h3p[:, :w])
                    else:
                        nc.vector.tensor_tensor(out=vmax[:, :w], in0=vmax[:, :w], in1=h3p[:, :w], op=ALU.max)
                        nc.vector.tensor_tensor(out=vmin[:, :w], in0=vmin[:, :w], in1=h3p[:, :w], op=ALU.min)

                # finalize tile: mean/max/min + b3, direct column-major writes
                mean_s = stagep.tile([lat_ch, W], F32, tag="mean_s")
                nc.vector.tensor_tensor(out=mean_s[:, :wt], in0=psum[:, :wt],
                                        in1=rdeg_sb[:, cc0 * P : cc0 * P + wt], op=ALU.mult)
                mean_f = stagep.tile([lat_ch, W], F32, tag="mean_f")
                nc.scalar.activation(mean_f[:, :wt], mean_s[:, :wt], AF.Identity, bias=b3[:])
                max_f = stagep.tile([lat_ch, W], F32, tag="max_f")
                nc.scalar.activation(max_f[:, :wt], vmax[:, :wt], AF.Identity, bias=b3[:])
                min_f = stagep.tile([lat_ch, W], F32, tag="min_f")
                nc.scalar.activation(min_f[:, :wt], vmin[:, :wt], AF.Identity, bias=b3[:])
                nc.sync.dma_start(outT_t[0:lat_ch, cc0 * P : cc0 * P + wt], mean_f[:, :wt])
                nc.sync.dma_start(outT_t[lat_ch : 2 * lat_ch, cc0 * P : cc0 * P + wt], max_f[:, :wt])
                nc.sync.dma_start(outT_t[2 * lat_ch : 3 * lat_ch, cc0 * P : cc0 * P + wt], min_f[:, :wt])
    return nc


# ---------------- public entry point ----------------

N_NODES = 50000
N_EDGES = 800000
IN_CH = 64
HID_CH = 128
LAT_CH = 64
N_GRAPHS = 64
U_DIM = 32
N_CORES = 8


def make_in_maps(sched, x, W1, W2, W3, b1, b2, b3):
    """Per-core input dicts (shared program, per-core data)."""
    x16 = x.astype(np.float16)
    ncc = sched["n_col_chunks"]
    col_node = sched["col_node"]
    col_pos = sched["col_pos"]
    deg_all = sched["deg_all"]
    n_cols = sched["n_cols"]
    ident = np.eye(P, dtype=np.float16)

    in_maps = []
    for c in range(N_CORES):
        nodes = col_node[c]  # [n_cols], -1 virtual
        real = nodes >= 0
        xcolT = np.zeros((64, ncc * P), np.float16)
        xcolT[:, col_pos[real]] = x16[nodes[real]].T
        rdeg = np.ones((1, ncc * P), np.float32)
        rdeg[0, col_pos[real]] = 1.0 / deg_all[nodes[real]]
        rdeg = np.broadcast_to(rdeg, (64, ncc * P)).copy()
        in_maps.append({
            "x16": x16,
            "idx_row": sched["idx_row"][c],
            "xcolT": xcolT, "rdeg": rdeg,
            "W1a": W1[:64].astype(np.float16), "W1b": W1[64:].astype(np.float16),
            "W2": W2.astype(np.float16), "W3": W3.astype(np.float16),
            "b1": np.ascontiguousarray(b1[:, None].astype(np.float32)),
            "b2": np.ascontiguousarray(b2[:, None].astype(np.float32)),
            "b3": np.ascontiguousarray(b3[:, None].astype(np.float32)),
            "ident": ident,
        })
    return in_maps


def assemble_output(sched, res_list, x, u, batch):
    """Un-permute per-core column-major results and build the full output."""
    n_nodes = x.shape[0]
    col_node = sched["col_node"]
    col_pos = sched["col_pos"]
    out = np.zeros((n_nodes, 288), np.float32)
    out[:, 0:64] = x
    out[:, 256:288] = u[batch]
    for c in range(N_CORES):
        outT = res_list[c]["outT"]  # [192, ncc*P]
        nodes = col_node[c]
        real = nodes >= 0
        out[nodes[real], 64:256] = outT[:, col_pos[real]].T
    return out


def kernel(**inputs):
    """Full-input NodeModel forward. Returns [N_NODES, 288] float32."""
    from concourse.bass_utils import run_bass_kernel_spmd

    x = np.asarray(inputs["x"], np.float32)
    edge_index = np.asarray(inputs["edge_index"])
    u = np.asarray(inputs["u"], np.float32)
    batch = np.asarray(inputs["batch"])
    W1 = np.asarray(inputs["W1"], np.float32)
    b1 = np.asarray(inputs["b1"], np.float32)
    W2 = np.asarray(inputs["W2"], np.float32)
    b2 = np.asarray(inputs["b2"], np.float32)
    W3 = np.asarray(inputs["W3"], np.float32)
    b3 = np.asarray(inputs["b3"], np.float32)

    row = edge_index[0].astype(np.int32)
    col = edge_index[1].astype(np.int32)

    sched = build_schedule(col, x.shape[0], N_CORES)
    sched["idx_row"] = fill_row_indices(sched, row, N_CORES)

    nc = build_kernel(sched, x.shape[0], W2.shape[0], W3.shape[1])
    in_maps = make_in_maps(sched, x, W1, W2, W3, b1, b2, b3)

    res = run_bass_kernel_spmd(nc, in_maps, core_ids=list(range(N_CORES)))
    return assemble_output(sched, res.results, x, u, batch).astype(np.float32)
